# revision 14
# baseline (speedup 1.0000x reference)
"""DGCNN on Trainium2: data-parallel over batch B=8 across 8 NeuronCores.

Per core (one point cloud, N=2048):
  Each EdgeConv layer (C -> O) is decomposed as
    y[n,m]   = <h_n, h_m> - ||h_m||^2/2          (monotone in -dist^2, row-wise)
    topk_20  = 3 rounds of DVE max8/max_index/match_replace over y rows
    u        = h @ Wd^T              (Wd = W[:, :C], the (nbr-center) half)
    v        = h @ (Wc - Wd)^T + b   (Wc = W[:, C:])
    h'       = LeakyReLU(v_n + max_{m in knn(n)} u_m)
  using that max over neighbors commutes with the (monotone) LeakyReLU and
  with the per-point additive term v_n.  Neighbor u columns are fetched from
  SBUF-resident uT with gpsimd ap_gather (per-16-partition wrapped indices),
  then max-reduced on DVE — results land directly in transposed (O, N)
  layout for the next layer.
  Final layer: out = max_n LeakyReLU(hcat @ Wf^T + bf)  (max commutes again).
"""

import numpy as np

import concourse.bass as bass
import concourse.mybir as mybir
import concourse.tile as tile

F32 = mybir.dt.float32
U16 = mybir.dt.uint16
I16 = mybir.dt.int16
AX = mybir.AxisListType
ALU = mybir.AluOpType
ACTF = mybir.ActivationFunctionType

B, N, K = 8, 2048, 20
P = 128
NT = N // P  # 16 row tiles
LAYERS = [(3, 64), (64, 64), (64, 128), (128, 256)]
CSUM = 512
FEAT = 1024
SLOPE = 0.2
NEG_SENTINEL = -3.4028234663852886e38  # -FLT_MAX
KPAD = 24  # 3 rounds of max8; slots 20..23 hold duplicates of 0..3
NIDX = P * KPAD  # 3072 gather indices per row tile


def build_kernel(gather_mode="ap"):
    from contextlib import ExitStack

    from concourse import bacc

    nc = bacc.Bacc("TRN2", target_bir_lowering=False, debug=False)

    xT = nc.dram_tensor("xT", (3, N), F32, kind="ExternalInput").ap()
    wd_d, wv_d, bias_d = [], [], []
    for li, (C, O) in enumerate(LAYERS):
        wd_d.append(nc.dram_tensor(f"wd{li}", (C, O), F32, kind="ExternalInput").ap())
        wv_d.append(nc.dram_tensor(f"wv{li}", (C, O), F32, kind="ExternalInput").ap())
        bias_d.append(
            nc.dram_tensor(f"bias{li}", (1, O), F32, kind="ExternalInput").ap()
        )
    wfT = nc.dram_tensor("wfT", (CSUM, FEAT), F32, kind="ExternalInput").ap()
    bf = nc.dram_tensor("bf", (1, FEAT), F32, kind="ExternalInput").ap()
    out = nc.dram_tensor("out", (P, FEAT // P), F32, kind="ExternalOutput").ap()

    with tile.TileContext(nc) as tc, ExitStack() as ctx:
        const = ctx.enter_context(tc.tile_pool(name="const", bufs=1))
        work = ctx.enter_context(tc.tile_pool(name="work", bufs=2))
        ypool = ctx.enter_context(tc.tile_pool(name="ypool", bufs=2))
        gpool = ctx.enter_context(tc.tile_pool(name="gpool", bufs=2))
        onepool = ctx.enter_context(tc.tile_pool(name="onepool", bufs=1))
        ypsum = ctx.enter_context(tc.tile_pool(name="ypsum", bufs=4, space="PSUM"))
        spsum = ctx.enter_context(tc.tile_pool(name="spsum", bufs=3, space="PSUM"))

        ones_col = const.tile([1, P], F32, tag="ones")
        nc.vector.memset(ones_col, 1.0)
        ones_row = const.tile([1, 512], F32, tag="onesr")
        nc.vector.memset(ones_row, 1.0)
        neghalf = const.tile([P, 1], F32, tag="neghalf")
        nc.vector.memset(neghalf, -0.5)

        # hcat layout: [128, 4, N]; channel c of hcat = hcat[c % 128, c // 128]
        hcat = const.tile([P, 4, N], F32, tag="hcat")
        xT_sb = const.tile([3, N], F32, tag="xT")
        nc.sync.dma_start(xT_sb, xT)
        wfT_sb = const.tile([P, 4, FEAT], F32, tag="wfT")
        nc.sync.dma_start(wfT_sb, wfT.rearrange("(t p) f -> p t f", p=P))
        bf_sb = const.tile([P, FEAT // P], F32, tag="bf")
        nc.sync.dma_start(bf_sb, bf.rearrange("o (t p) -> p (o t)", p=P))

        # h2 needs its own base-partition-0 tensor (PE matmul operands must
        # share base partition); it is DMA'd into hcat[64:128, 0] at the end.
        h2T_sb = const.tile([64, N], F32, tag="h2T")
        h_ins = [xT_sb, hcat[0:64, 0], h2T_sb, hcat[:, 1]]
        # per layer: list of (o0, o1, dest (O-chunk, N) AP) for the output
        h_outs = [
            [(0, 64, hcat[0:64, 0])],
            [(0, 64, h2T_sb)],
            [(0, 128, hcat[:, 1])],
            [(0, 128, hcat[:, 2]), (128, 256, hcat[:, 3])],
        ]

        for li, (C, O) in enumerate(LAYERS):
            hT = h_ins[li]  # (C, N)
            wd_sb = const.tile([C, O], F32, tag=f"wd{li}")
            nc.sync.dma_start(wd_sb, wd_d[li])
            wv_sb = const.tile([C, O], F32, tag=f"wv{li}")
            nc.sync.dma_start(wv_sb, wv_d[li])
            bias_sb = const.tile([1, O], F32, tag=f"bias{li}")
            nc.sync.dma_start(bias_sb, bias_d[li])

            # negxx[m] = -||h_m||^2 / 2, as a (1, N) row
            hsq = onepool.tile([C, N], F32, tag="hsq")
            nc.vector.tensor_tensor(hsq, hT, hT, op=ALU.mult)
            negxx = onepool.tile([1, N], F32, tag="negxx")
            for j in range(4):
                ps = spsum.tile([1, 512], F32, tag="sp")
                nc.tensor.matmul(
                    out=ps,
                    lhsT=neghalf[:C],
                    rhs=hsq[:, j * 512 : (j + 1) * 512],
                    start=True,
                    stop=True,
                )
                nc.scalar.copy(negxx[:, j * 512 : (j + 1) * 512], ps)

            # uT/vT: (O, N) SBUF, in <=128-row chunks; vT includes the bias
            chunks = h_outs[li]
            uT = []
            vT = []
            for ci, (o0, o1, _dest) in enumerate(chunks):
                W = o1 - o0
                uTc = onepool.tile([W, N], F32, tag=f"uT{ci}")
                vTc = onepool.tile([W, N], F32, tag=f"vT{ci}")
                for j in range(4):
                    msl = slice(j * 512, (j + 1) * 512)
                    psu = spsum.tile([W, 512], F32, tag="sp")
                    nc.tensor.matmul(
                        out=psu, lhsT=wd_sb[:, o0:o1], rhs=hT[:, msl],
                        start=True, stop=True,
                    )
                    nc.scalar.copy(uTc[:, msl], psu)
                    psv = spsum.tile([W, 512], F32, tag="sp")
                    nc.tensor.matmul(
                        out=psv, lhsT=wv_sb[:, o0:o1], rhs=hT[:, msl],
                        start=True, stop=False,
                    )
                    nc.tensor.matmul(
                        out=psv, lhsT=bias_sb[:, o0:o1], rhs=ones_row,
                        start=False, stop=True,
                    )
                    nc.scalar.copy(vTc[:, msl], psv)
                uT.append(uTc)
                vT.append(vTc)

            for t in range(NT):
                nsl = slice(t * P, (t + 1) * P)
                # y[n, m] = <h_n, h_m> - xx_m/2 for the 128 rows of this tile
                ybanks = []
                for j in range(4):
                    msl = slice(j * 512, (j + 1) * 512)
                    yb = ypsum.tile([P, 512], F32, tag="yb")
                    nc.tensor.matmul(
                        out=yb, lhsT=hT[:, nsl], rhs=hT[:, msl],
                        start=True, stop=False,
                    )
                    nc.tensor.matmul(
                        out=yb, lhsT=ones_col, rhs=negxx[:, msl],
                        start=False, stop=True,
                    )
                    ybanks.append(yb)
                y_sb = ypool.tile([P, N], F32, tag="y")
                for j in range(4):
                    nc.scalar.copy(y_sb[:, j * 512 : (j + 1) * 512], ybanks[j])

                # top-20 (+4 padding) neighbor indices by descending y
                idx24 = work.tile([P, KPAD], U16, tag="idx")
                for r in range(3):
                    vm = work.tile([P, 8], F32, tag="vmax")
                    nc.vector.max(out=vm, in_=y_sb)
                    nc.vector.max_index(
                        out=idx24[:, r * 8 : (r + 1) * 8], in_max=vm, in_values=y_sb
                    )
                    if r < 2:
                        nc.vector.match_replace(
                            out=y_sb, in_to_replace=vm, in_values=y_sb,
                            imm_value=NEG_SENTINEL,
                        )
                # ranks 21..24 -> duplicates of ranks 1..4 (harmless under max)
                nc.vector.tensor_copy(idx24[:, 20:24], idx24[:, 0:4])

                # wrap indices for ap_gather: flat index i = c*128 + n, stored
                # at [partition i%16, slot i//16]; replicated to all 8 Q7
                # core groups.
                wrapped = work.tile([P, KPAD * 8], I16, tag="wrap")
                idx_i16 = idx24.bitcast(I16)
                w3 = wrapped.rearrange("p (c r) -> p c r", r=8)
                for r in range(8):
                    nc.sync.dma_start(
                        w3[0:16, :, r], idx_i16[16 * r : 16 * (r + 1), :]
                    )
                for g in range(1, 8):
                    nc.gpsimd.dma_start(
                        wrapped[16 * g : 16 * (g + 1), :], wrapped[0:16, :]
                    )

                # gather + neighbor-max per O-chunk; output lands transposed
                for ci, (o0, o1, dest) in enumerate(chunks):
                    W = o1 - o0
                    gath = gpool.tile([P, NIDX], F32, tag="gath")
                    if gather_mode == "ap":
                        nc.gpsimd.ap_gather(
                            out_ap=gath[:W],
                            in_ap=uT[ci],
                            idxs_ap=wrapped[:W],
                            channels=W,
                            num_elems=N,
                            d=1,
                            num_idxs=NIDX,
                        )
                    else:  # bisect stub
                        nc.vector.memset(gath[:W], 0.0)
                    # gath[o, c*128 + n] -> reduce over c
                    red = work.tile([P, P], F32, tag="red")
                    nc.vector.tensor_reduce(
                        out=red[:W],
                        in_=gath[:W].rearrange("o (c n) -> o n c", n=P),
                        axis=AX.X,
                        op=ALU.max,
                    )
                    nc.vector.tensor_add(red[:W], red[:W], vT[ci][:, nsl])
                    tmp = work.tile([P, P], F32, tag="tmp")
                    nc.scalar.activation(tmp[:W], red[:W], ACTF.Copy, scale=SLOPE)
                    nc.vector.tensor_tensor(
                        dest[:, nsl], red[:W], tmp[:W], op=ALU.max
                    )

        # assemble h2 into hcat (partition-crossing move -> DMA)
        nc.sync.dma_start(hcat[64:128, 0], h2T_sb)

        # final: out[f] = LeakyReLU(bf[f] + max_n (hcat @ WfT)[n, f])
        for ft in range(FEAT // P):
            fsl = slice(ft * P, (ft + 1) * P)
            fm4 = work.tile([P, 4], F32, tag="fm")
            for nj in range(4):
                msl = slice(nj * 512, (nj + 1) * 512)
                pf = spsum.tile([P, 512], F32, tag="sp")
                for ct in range(4):
                    nc.tensor.matmul(
                        out=pf,
                        lhsT=wfT_sb[:, ct, fsl],
                        rhs=hcat[:, ct, msl],
                        start=(ct == 0),
                        stop=(ct == 3),
                    )
                nc.vector.tensor_reduce(
                    out=fm4[:, nj : nj + 1], in_=pf, axis=AX.X, op=ALU.max
                )
            m1 = work.tile([P, 1], F32, tag="m1")
            nc.vector.tensor_reduce(out=m1, in_=fm4, axis=AX.X, op=ALU.max)
            nc.vector.tensor_add(m1, m1, bf_sb[:, ft : ft + 1])
            t2 = work.tile([P, 1], F32, tag="t2")
            nc.scalar.activation(t2, m1, ACTF.Copy, scale=SLOPE)
            oc = work.tile([P, 1], F32, tag="oc")
            nc.vector.tensor_tensor(oc, m1, t2, op=ALU.max)
            nc.sync.dma_start(out[:, ft : ft + 1], oc)

    nc.compile()
    return nc


def make_in_maps(x, W1, b1, W2, b2, W3, b3, W4, b4, Wf, bf):
    """Host-side prep: shard x over batch, pre-transpose weights."""
    f32 = np.float32
    Ws = [W1, W2, W3, W4]
    bs = [b1, b2, b3, b4]
    shared = {}
    for li, (C, O) in enumerate(LAYERS):
        W = np.asarray(Ws[li], f32)
        Wd = W[:, :C]
        Wc = W[:, C:]
        shared[f"wd{li}"] = np.ascontiguousarray(Wd.T)
        shared[f"wv{li}"] = np.ascontiguousarray((Wc - Wd).T)
        shared[f"bias{li}"] = np.ascontiguousarray(np.asarray(bs[li], f32)[None, :])
    shared["wfT"] = np.ascontiguousarray(np.asarray(Wf, f32).T)
    shared["bf"] = np.ascontiguousarray(np.asarray(bf, f32).reshape(1, FEAT))
    in_maps = []
    for b in range(B):
        m = dict(shared)
        m["xT"] = np.ascontiguousarray(np.asarray(x[b], f32).T)
        in_maps.append(m)
    return in_maps


_NC_CACHE = {}


def kernel(x, W1, b1, W2, b2, W3, b3, W4, b4, Wf, bf):
    from concourse.bass_utils import run_bass_kernel_spmd

    if "nc" not in _NC_CACHE:
        _NC_CACHE["nc"] = build_kernel()
    nc = _NC_CACHE["nc"]
    in_maps = make_in_maps(x, W1, b1, W2, b2, W3, b3, W4, b4, Wf, bf)
    res = run_bass_kernel_spmd(nc, in_maps, core_ids=list(range(B)))
    out = np.stack([r["out"].T.ravel() for r in res.results])
    return out.astype(np.float32)


# revision 17
# speedup vs baseline: 404.4593x; 404.4593x over previous
"""DGCNN on Trainium2: data-parallel over batch B=8 across 8 NeuronCores.

Per core (one point cloud, N=2048):
  Each EdgeConv layer (C -> O) is decomposed as
    y[n,m]   = <h_n, h_m> - ||h_m||^2/2          (monotone in -dist^2, row-wise)
    topk_20  = 3 rounds of DVE max8/max_index/match_replace over y rows
    u        = h @ Wd^T              (Wd = W[:, :C], the (nbr-center) half)
    v        = h @ (Wc - Wd)^T + b   (Wc = W[:, C:])
    h'       = LeakyReLU(v_n + max_{m in knn(n)} u_m)
  using that max over neighbors commutes with the (monotone) LeakyReLU and
  with the per-point additive term v_n.  Neighbor u columns are fetched from
  SBUF-resident uT with gpsimd ap_gather (per-16-partition wrapped indices),
  then max-reduced on DVE — results land directly in transposed (O, N)
  layout for the next layer.
  Final layer: out = max_n LeakyReLU(hcat @ Wf^T + bf)  (max commutes again).
"""

import numpy as np

import concourse.bass as bass
import concourse.mybir as mybir
import concourse.tile as tile

F32 = mybir.dt.float32
U16 = mybir.dt.uint16
I16 = mybir.dt.int16
AX = mybir.AxisListType
ALU = mybir.AluOpType
ACTF = mybir.ActivationFunctionType

B, N, K = 8, 2048, 20
P = 128
NT = N // P  # 16 row tiles
LAYERS = [(3, 64), (64, 64), (64, 128), (128, 256)]
CSUM = 512
FEAT = 1024
SLOPE = 0.2
NEG_SENTINEL = -3.4028234663852886e38  # -FLT_MAX
KPAD = 24  # 3 rounds of max8; slots 20..23 hold duplicates of 0..3
NIDX = P * KPAD  # 3072 gather indices per row tile


def build_kernel(gather_mode="ap"):
    from contextlib import ExitStack

    from concourse import bacc

    nc = bacc.Bacc("TRN2", target_bir_lowering=False, debug=False)

    xT = nc.dram_tensor("xT", (3, N), F32, kind="ExternalInput").ap()
    wd_d, wv_d, bias_d = [], [], []
    for li, (C, O) in enumerate(LAYERS):
        wd_d.append(nc.dram_tensor(f"wd{li}", (C, O), F32, kind="ExternalInput").ap())
        wv_d.append(nc.dram_tensor(f"wv{li}", (C, O), F32, kind="ExternalInput").ap())
        bias_d.append(
            nc.dram_tensor(f"bias{li}", (1, O), F32, kind="ExternalInput").ap()
        )
    wfT = nc.dram_tensor("wfT", (CSUM, FEAT), F32, kind="ExternalInput").ap()
    bf = nc.dram_tensor("bf", (1, FEAT), F32, kind="ExternalInput").ap()
    out = nc.dram_tensor("out", (P, FEAT // P), F32, kind="ExternalOutput").ap()

    with tile.TileContext(nc) as tc, ExitStack() as ctx:
        const = ctx.enter_context(tc.tile_pool(name="const", bufs=1))
        work = ctx.enter_context(tc.tile_pool(name="work", bufs=3))
        ypool = ctx.enter_context(tc.tile_pool(name="ypool", bufs=3))
        gpool = ctx.enter_context(tc.tile_pool(name="gpool", bufs=3))
        onepool = ctx.enter_context(tc.tile_pool(name="onepool", bufs=1))
        ypsum = ctx.enter_context(tc.tile_pool(name="ypsum", bufs=4, space="PSUM"))
        spsum = ctx.enter_context(tc.tile_pool(name="spsum", bufs=3, space="PSUM"))

        ones_col = const.tile([1, P], F32, tag="ones")
        nc.vector.memset(ones_col, 1.0)
        ones_row = const.tile([1, 512], F32, tag="onesr")
        nc.vector.memset(ones_row, 1.0)
        neghalf = const.tile([P, 1], F32, tag="neghalf")
        nc.vector.memset(neghalf, -0.5)

        # hcat layout: [128, 4, N]; channel c of hcat = hcat[c % 128, c // 128]
        hcat = const.tile([P, 4, N], F32, tag="hcat")
        xT_sb = const.tile([3, N], F32, tag="xT")
        nc.sync.dma_start(xT_sb, xT)
        wfT_sb = const.tile([P, 4, FEAT], F32, tag="wfT")
        nc.sync.dma_start(wfT_sb, wfT.rearrange("(t p) f -> p t f", p=P))
        bf_sb = const.tile([P, FEAT // P], F32, tag="bf")
        nc.sync.dma_start(bf_sb, bf.rearrange("o (t p) -> p (o t)", p=P))

        # h2 needs its own base-partition-0 tensor (PE matmul operands must
        # share base partition); it is DMA'd into hcat[64:128, 0] at the end.
        h2T_sb = const.tile([64, N], F32, tag="h2T")
        h_ins = [xT_sb, hcat[0:64, 0], h2T_sb, hcat[:, 1]]
        # per layer: list of (o0, o1, dest (O-chunk, N) AP) for the output
        h_outs = [
            [(0, 64, hcat[0:64, 0])],
            [(0, 64, h2T_sb)],
            [(0, 128, hcat[:, 1])],
            [(0, 128, hcat[:, 2]), (128, 256, hcat[:, 3])],
        ]

        for li, (C, O) in enumerate(LAYERS):
            hT = h_ins[li]  # (C, N)
            wd_sb = const.tile([C, O], F32, tag=f"wd{li}")
            nc.sync.dma_start(wd_sb, wd_d[li])
            wv_sb = const.tile([C, O], F32, tag=f"wv{li}")
            nc.sync.dma_start(wv_sb, wv_d[li])
            bias_sb = const.tile([1, O], F32, tag=f"bias{li}")
            nc.sync.dma_start(bias_sb, bias_d[li])

            # negxx[m] = -||h_m||^2 / 2, as a (1, N) row
            hsq = onepool.tile([C, N], F32, tag="hsq")
            nc.vector.tensor_tensor(hsq, hT, hT, op=ALU.mult)
            negxx = onepool.tile([1, N], F32, tag="negxx")
            for j in range(4):
                ps = spsum.tile([1, 512], F32, tag="sp")
                nc.tensor.matmul(
                    out=ps,
                    lhsT=neghalf[:C],
                    rhs=hsq[:, j * 512 : (j + 1) * 512],
                    start=True,
                    stop=True,
                )
                nc.scalar.copy(negxx[:, j * 512 : (j + 1) * 512], ps)

            # uT/vT: (O, N) SBUF, in <=128-row chunks; vT includes the bias
            chunks = h_outs[li]
            uT = []
            vT = []
            for ci, (o0, o1, _dest) in enumerate(chunks):
                W = o1 - o0
                uTc = onepool.tile([W, N], F32, tag=f"uT{ci}")
                vTc = onepool.tile([W, N], F32, tag=f"vT{ci}")
                for j in range(4):
                    msl = slice(j * 512, (j + 1) * 512)
                    psu = spsum.tile([W, 512], F32, tag="sp")
                    nc.tensor.matmul(
                        out=psu, lhsT=wd_sb[:, o0:o1], rhs=hT[:, msl],
                        start=True, stop=True,
                    )
                    nc.scalar.copy(uTc[:, msl], psu)
                    psv = spsum.tile([W, 512], F32, tag="sp")
                    nc.tensor.matmul(
                        out=psv, lhsT=wv_sb[:, o0:o1], rhs=hT[:, msl],
                        start=True, stop=False,
                    )
                    nc.tensor.matmul(
                        out=psv, lhsT=bias_sb[:, o0:o1], rhs=ones_row,
                        start=False, stop=True,
                    )
                    nc.scalar.copy(vTc[:, msl], psv)
                uT.append(uTc)
                vT.append(vTc)

            for t in range(NT):
                nsl = slice(t * P, (t + 1) * P)
                # y[n, m] = <h_n, h_m> - xx_m/2 for the 128 rows of this tile
                ybanks = []
                for j in range(4):
                    msl = slice(j * 512, (j + 1) * 512)
                    yb = ypsum.tile([P, 512], F32, tag="yb")
                    nc.tensor.matmul(
                        out=yb, lhsT=hT[:, nsl], rhs=hT[:, msl],
                        start=True, stop=False,
                    )
                    nc.tensor.matmul(
                        out=yb, lhsT=ones_col, rhs=negxx[:, msl],
                        start=False, stop=True,
                    )
                    ybanks.append(yb)
                y_sb = ypool.tile([P, N], F32, tag="y")
                for j in range(4):
                    nc.scalar.copy(y_sb[:, j * 512 : (j + 1) * 512], ybanks[j])

                # top-20 (+4 padding) neighbor indices by descending y
                idx24 = work.tile([P, KPAD], U16, tag="idx")
                for r in range(3):
                    vm = work.tile([P, 8], F32, tag="vmax")
                    nc.vector.max(out=vm, in_=y_sb)
                    nc.vector.max_index(
                        out=idx24[:, r * 8 : (r + 1) * 8], in_max=vm, in_values=y_sb
                    )
                    if r < 2:
                        nc.vector.match_replace(
                            out=y_sb, in_to_replace=vm, in_values=y_sb,
                            imm_value=NEG_SENTINEL,
                        )
                # ranks 21..24 -> duplicates of ranks 1..4 (harmless under max)
                nc.vector.tensor_copy(idx24[:, 20:24], idx24[:, 0:4])

                # wrap indices for ap_gather: flat index i = c*128 + n, stored
                # at [partition i%16, slot i//16]; replicated to all 8 Q7
                # core groups.
                wrapped = work.tile([P, KPAD * 8], I16, tag="wrap")
                if gather_mode == "zeroidx":
                    nc.vector.memset(wrapped, 0)
                else:
                    idx_i16 = idx24.bitcast(I16)
                    w3 = wrapped.rearrange("p (c r) -> p c r", r=8)
                    for r in range(8):
                        nc.sync.dma_start(
                            w3[0:16, :, r], idx_i16[16 * r : 16 * (r + 1), :]
                        )
                    # replicate to the Q7 core groups the gather will use
                    ngroups = max((o1 - o0) for (o0, o1, _d) in chunks) // 16
                    for g in range(1, ngroups):
                        nc.gpsimd.dma_start(
                            wrapped[16 * g : 16 * (g + 1), :], wrapped[0:16, :]
                        )

                # gather + neighbor-max per O-chunk; output lands transposed
                for ci, (o0, o1, dest) in enumerate(chunks):
                    W = o1 - o0
                    gath = gpool.tile([P, NIDX], F32, tag="gath")
                    if gather_mode == "ap":
                        nc.gpsimd.ap_gather(
                            out_ap=gath[:W],
                            in_ap=uT[ci],
                            idxs_ap=wrapped[:W],
                            channels=W,
                            num_elems=N,
                            d=1,
                            num_idxs=NIDX,
                        )
                    else:  # bisect stub
                        nc.vector.memset(gath[:W], 0.0)
                    # gath[o, c*128 + n] -> reduce over c
                    red = work.tile([P, P], F32, tag="red")
                    nc.vector.tensor_reduce(
                        out=red[:W],
                        in_=gath[:W].rearrange("o (c n) -> o n c", n=P),
                        axis=AX.X,
                        op=ALU.max,
                    )
                    nc.vector.tensor_add(red[:W], red[:W], vT[ci][:, nsl])
                    tmp = work.tile([P, P], F32, tag="tmp")
                    nc.scalar.activation(tmp[:W], red[:W], ACTF.Copy, scale=SLOPE)
                    nc.vector.tensor_tensor(
                        dest[:, nsl], red[:W], tmp[:W], op=ALU.max
                    )

        # assemble h2 into hcat (partition-crossing move -> DMA)
        nc.sync.dma_start(hcat[64:128, 0], h2T_sb)

        # final: out[f] = LeakyReLU(bf[f] + max_n (hcat @ WfT)[n, f])
        for ft in range(FEAT // P):
            fsl = slice(ft * P, (ft + 1) * P)
            fm4 = work.tile([P, 4], F32, tag="fm")
            for nj in range(4):
                msl = slice(nj * 512, (nj + 1) * 512)
                pf = spsum.tile([P, 512], F32, tag="sp")
                for ct in range(4):
                    nc.tensor.matmul(
                        out=pf,
                        lhsT=wfT_sb[:, ct, fsl],
                        rhs=hcat[:, ct, msl],
                        start=(ct == 0),
                        stop=(ct == 3),
                    )
                nc.vector.tensor_reduce(
                    out=fm4[:, nj : nj + 1], in_=pf, axis=AX.X, op=ALU.max
                )
            m1 = work.tile([P, 1], F32, tag="m1")
            nc.vector.tensor_reduce(out=m1, in_=fm4, axis=AX.X, op=ALU.max)
            nc.vector.tensor_add(m1, m1, bf_sb[:, ft : ft + 1])
            t2 = work.tile([P, 1], F32, tag="t2")
            nc.scalar.activation(t2, m1, ACTF.Copy, scale=SLOPE)
            oc = work.tile([P, 1], F32, tag="oc")
            nc.vector.tensor_tensor(oc, m1, t2, op=ALU.max)
            nc.sync.dma_start(out[:, ft : ft + 1], oc)

    nc.compile()
    return nc


def make_in_maps(x, W1, b1, W2, b2, W3, b3, W4, b4, Wf, bf):
    """Host-side prep: shard x over batch, pre-transpose weights."""
    f32 = np.float32
    Ws = [W1, W2, W3, W4]
    bs = [b1, b2, b3, b4]
    shared = {}
    for li, (C, O) in enumerate(LAYERS):
        W = np.asarray(Ws[li], f32)
        Wd = W[:, :C]
        Wc = W[:, C:]
        shared[f"wd{li}"] = np.ascontiguousarray(Wd.T)
        shared[f"wv{li}"] = np.ascontiguousarray((Wc - Wd).T)
        shared[f"bias{li}"] = np.ascontiguousarray(np.asarray(bs[li], f32)[None, :])
    shared["wfT"] = np.ascontiguousarray(np.asarray(Wf, f32).T)
    shared["bf"] = np.ascontiguousarray(np.asarray(bf, f32).reshape(1, FEAT))
    in_maps = []
    for b in range(B):
        m = dict(shared)
        m["xT"] = np.ascontiguousarray(np.asarray(x[b], f32).T)
        in_maps.append(m)
    return in_maps


_NC_CACHE = {}


def kernel(x, W1, b1, W2, b2, W3, b3, W4, b4, Wf, bf):
    from concourse.bass_utils import run_bass_kernel_spmd

    if "nc" not in _NC_CACHE:
        _NC_CACHE["nc"] = build_kernel()
    nc = _NC_CACHE["nc"]
    in_maps = make_in_maps(x, W1, b1, W2, b2, W3, b3, W4, b4, Wf, bf)
    res = run_bass_kernel_spmd(nc, in_maps, core_ids=list(range(B)))
    out = np.stack([r["out"].T.ravel() for r in res.results])
    return out.astype(np.float32)


# revision 20
# speedup vs baseline: 407.1688x; 1.0067x over previous
"""DGCNN on Trainium2: data-parallel over batch B=8 across 8 NeuronCores.

Per core (one point cloud, N=2048):
  Each EdgeConv layer (C -> O) is decomposed as
    y[n,m]   = <h_n, h_m> - ||h_m||^2/2          (monotone in -dist^2, row-wise)
    topk_20  = 3 rounds of DVE max8/max_index/match_replace over y rows
    u        = h @ Wd^T              (Wd = W[:, :C], the (nbr-center) half)
    v        = h @ (Wc - Wd)^T + b   (Wc = W[:, C:])
    h'       = LeakyReLU(v_n + max_{m in knn(n)} u_m)
  using that max over neighbors commutes with the (monotone) LeakyReLU and
  with the per-point additive term v_n.  Neighbor u columns are fetched from
  SBUF-resident uT with gpsimd ap_gather (per-16-partition wrapped indices),
  then max-reduced on DVE — results land directly in transposed (O, N)
  layout for the next layer.
  Final layer: out = max_n LeakyReLU(hcat @ Wf^T + bf)  (max commutes again).
"""

import numpy as np

import concourse.bass as bass
import concourse.mybir as mybir
import concourse.tile as tile

F32 = mybir.dt.float32
U16 = mybir.dt.uint16
I16 = mybir.dt.int16
AX = mybir.AxisListType
ALU = mybir.AluOpType
ACTF = mybir.ActivationFunctionType

B, N, K = 8, 2048, 20
P = 128
NT = N // P  # 16 row tiles
LAYERS = [(3, 64), (64, 64), (64, 128), (128, 256)]
CSUM = 512
FEAT = 1024
SLOPE = 0.2
NEG_SENTINEL = -3.4028234663852886e38  # -FLT_MAX
NIDX = P * K  # 2560 gather indices per row tile (ap_gather only needs %4==0)


def build_kernel(gather_mode="ap"):
    from contextlib import ExitStack

    from concourse import bacc

    nc = bacc.Bacc("TRN2", target_bir_lowering=False, debug=False)

    xT = nc.dram_tensor("xT", (3, N), F32, kind="ExternalInput").ap()
    wd_d, wv_d, bias_d = [], [], []
    for li, (C, O) in enumerate(LAYERS):
        wd_d.append(nc.dram_tensor(f"wd{li}", (C, O), F32, kind="ExternalInput").ap())
        wv_d.append(nc.dram_tensor(f"wv{li}", (C, O), F32, kind="ExternalInput").ap())
        bias_d.append(
            nc.dram_tensor(f"bias{li}", (1, O), F32, kind="ExternalInput").ap()
        )
    wfT = nc.dram_tensor("wfT", (CSUM, FEAT), F32, kind="ExternalInput").ap()
    bf = nc.dram_tensor("bf", (1, FEAT), F32, kind="ExternalInput").ap()
    out = nc.dram_tensor("out", (P, FEAT // P), F32, kind="ExternalOutput").ap()

    with tile.TileContext(nc) as tc, ExitStack() as ctx:
        const = ctx.enter_context(tc.tile_pool(name="const", bufs=1))
        work = ctx.enter_context(tc.tile_pool(name="work", bufs=3))
        ypool = ctx.enter_context(tc.tile_pool(name="ypool", bufs=3))
        gpool = ctx.enter_context(tc.tile_pool(name="gpool", bufs=4))
        onepool = ctx.enter_context(tc.tile_pool(name="onepool", bufs=1))
        ypsum = ctx.enter_context(tc.tile_pool(name="ypsum", bufs=4, space="PSUM"))
        spsum = ctx.enter_context(tc.tile_pool(name="spsum", bufs=3, space="PSUM"))

        ones_col = const.tile([1, P], F32, tag="ones")
        nc.vector.memset(ones_col, 1.0)
        ones_row = const.tile([1, 512], F32, tag="onesr")
        nc.vector.memset(ones_row, 1.0)
        neghalf = const.tile([P, 1], F32, tag="neghalf")
        nc.vector.memset(neghalf, -0.5)

        # hcat layout: [128, 4, N]; channel c of hcat = hcat[c % 128, c // 128]
        hcat = const.tile([P, 4, N], F32, tag="hcat")
        xT_sb = const.tile([3, N], F32, tag="xT")
        nc.sync.dma_start(xT_sb, xT)
        wfT_sb = const.tile([P, 4, FEAT], F32, tag="wfT")
        nc.sync.dma_start(wfT_sb, wfT.rearrange("(t p) f -> p t f", p=P))
        bf_sb = const.tile([P, FEAT // P], F32, tag="bf")
        nc.sync.dma_start(bf_sb, bf.rearrange("o (t p) -> p (o t)", p=P))

        # h2 needs its own base-partition-0 tensor (PE matmul operands must
        # share base partition); it is DMA'd into hcat[64:128, 0] at the end.
        h2T_sb = const.tile([64, N], F32, tag="h2T")
        h_ins = [xT_sb, hcat[0:64, 0], h2T_sb, hcat[:, 1]]
        # per layer: list of (o0, o1, dest (O-chunk, N) AP) for the output
        h_outs = [
            [(0, 64, hcat[0:64, 0])],
            [(0, 64, h2T_sb)],
            [(0, 128, hcat[:, 1])],
            [(0, 128, hcat[:, 2]), (128, 256, hcat[:, 3])],
        ]

        for li, (C, O) in enumerate(LAYERS):
            hT = h_ins[li]  # (C, N)
            wd_sb = const.tile([C, O], F32, tag=f"wd{li}")
            nc.sync.dma_start(wd_sb, wd_d[li])
            wv_sb = const.tile([C, O], F32, tag=f"wv{li}")
            nc.sync.dma_start(wv_sb, wv_d[li])
            bias_sb = const.tile([1, O], F32, tag=f"bias{li}")
            nc.sync.dma_start(bias_sb, bias_d[li])

            # negxx[m] = -||h_m||^2 / 2, as a (1, N) row
            hsq = onepool.tile([C, N], F32, tag="hsq")
            nc.vector.tensor_tensor(hsq, hT, hT, op=ALU.mult)
            negxx = onepool.tile([1, N], F32, tag="negxx")
            for j in range(4):
                ps = spsum.tile([1, 512], F32, tag="sp")
                nc.tensor.matmul(
                    out=ps,
                    lhsT=neghalf[:C],
                    rhs=hsq[:, j * 512 : (j + 1) * 512],
                    start=True,
                    stop=True,
                )
                nc.scalar.copy(negxx[:, j * 512 : (j + 1) * 512], ps)

            # uT/vT: (O, N) SBUF, in <=128-row chunks; vT includes the bias
            chunks = h_outs[li]
            uT = []
            vT = []
            for ci, (o0, o1, _dest) in enumerate(chunks):
                W = o1 - o0
                uTc = onepool.tile([W, N], F32, tag=f"uT{ci}")
                vTc = onepool.tile([W, N], F32, tag=f"vT{ci}")
                for j in range(4):
                    msl = slice(j * 512, (j + 1) * 512)
                    psu = spsum.tile([W, 512], F32, tag="sp")
                    nc.tensor.matmul(
                        out=psu, lhsT=wd_sb[:, o0:o1], rhs=hT[:, msl],
                        start=True, stop=True,
                    )
                    nc.scalar.copy(uTc[:, msl], psu)
                    psv = spsum.tile([W, 512], F32, tag="sp")
                    nc.tensor.matmul(
                        out=psv, lhsT=wv_sb[:, o0:o1], rhs=hT[:, msl],
                        start=True, stop=False,
                    )
                    nc.tensor.matmul(
                        out=psv, lhsT=bias_sb[:, o0:o1], rhs=ones_row,
                        start=False, stop=True,
                    )
                    nc.scalar.copy(vTc[:, msl], psv)
                uT.append(uTc)
                vT.append(vTc)

            def emit_reduce(t, gaths):
                """Neighbor-max + v + LeakyReLU for tile t (gaths: per chunk)."""
                nsl = slice(t * P, (t + 1) * P)
                for ci, (o0, o1, dest) in enumerate(chunks):
                    W = o1 - o0
                    gath = gaths[ci]
                    # gath[o, c*128 + n] -> reduce over c
                    red = work.tile([P, P], F32, tag="red")
                    nc.vector.tensor_reduce(
                        out=red[:W],
                        in_=gath[:W].rearrange("o (c n) -> o n c", n=P),
                        axis=AX.X,
                        op=ALU.max,
                    )
                    nc.vector.tensor_add(red[:W], red[:W], vT[ci][:, nsl])
                    tmp = work.tile([P, P], F32, tag="tmp")
                    nc.scalar.activation(tmp[:W], red[:W], ACTF.Copy, scale=SLOPE)
                    nc.vector.tensor_tensor(
                        dest[:, nsl], red[:W], tmp[:W], op=ALU.max
                    )

            pending = None  # deferred reduce stage: DVE consumes gath one tile late
            for t in range(NT):
                nsl = slice(t * P, (t + 1) * P)
                # y[n, m] = <h_n, h_m> - xx_m/2 for the 128 rows of this tile
                ybanks = []
                for j in range(4):
                    msl = slice(j * 512, (j + 1) * 512)
                    yb = ypsum.tile([P, 512], F32, tag="yb")
                    nc.tensor.matmul(
                        out=yb, lhsT=hT[:, nsl], rhs=hT[:, msl],
                        start=True, stop=False,
                    )
                    nc.tensor.matmul(
                        out=yb, lhsT=ones_col, rhs=negxx[:, msl],
                        start=False, stop=True,
                    )
                    ybanks.append(yb)
                y_sb = ypool.tile([P, N], F32, tag="y")
                for j in range(4):
                    nc.scalar.copy(y_sb[:, j * 512 : (j + 1) * 512], ybanks[j])

                # top-20 neighbor indices by descending y (3 rounds of max8)
                idx24 = work.tile([P, 24], U16, tag="idx")
                for r in range(3):
                    vm = work.tile([P, 8], F32, tag="vmax")
                    nc.vector.max(out=vm, in_=y_sb)
                    nc.vector.max_index(
                        out=idx24[:, r * 8 : (r + 1) * 8], in_max=vm, in_values=y_sb
                    )
                    if r < 2:
                        nc.vector.match_replace(
                            out=y_sb, in_to_replace=vm, in_values=y_sb,
                            imm_value=NEG_SENTINEL,
                        )

                # wrap first 20 indices for ap_gather: flat index i = c*128 + n,
                # stored at [partition i%16, slot i//16]; replicated to the Q7
                # core groups the gather uses.
                wrapped = work.tile([P, K * 8], I16, tag="wrap")
                if gather_mode == "zeroidx":
                    nc.vector.memset(wrapped, 0)
                else:
                    idx_i16 = idx24.bitcast(I16)
                    w3 = wrapped.rearrange("p (c r) -> p c r", r=8)
                    for r in range(8):
                        nc.sync.dma_start(
                            w3[0:16, :, r], idx_i16[16 * r : 16 * (r + 1), 0:K]
                        )
                    ngroups = max((o1 - o0) for (o0, o1, _d) in chunks) // 16
                    for g in range(1, ngroups):
                        nc.gpsimd.dma_start(
                            wrapped[16 * g : 16 * (g + 1), :], wrapped[0:16, :]
                        )

                # gather u columns of the 20 neighbors per chunk (Pool engine)
                gaths = []
                for ci, (o0, o1, _dest) in enumerate(chunks):
                    W = o1 - o0
                    gath = gpool.tile([P, NIDX], F32, tag="gath")
                    if gather_mode == "ap":
                        nc.gpsimd.ap_gather(
                            out_ap=gath[:W],
                            in_ap=uT[ci],
                            idxs_ap=wrapped[:W],
                            channels=W,
                            num_elems=N,
                            d=1,
                            num_idxs=NIDX,
                        )
                    else:  # bisect stub
                        nc.vector.memset(gath[:W], 0.0)
                    gaths.append(gath)

                # reduce stage runs one tile behind so the DVE never waits on
                # an in-flight gather at the head of its in-order queue
                if pending is not None:
                    emit_reduce(*pending)
                pending = (t, gaths)
            emit_reduce(*pending)

        # assemble h2 into hcat (partition-crossing move -> DMA)
        nc.sync.dma_start(hcat[64:128, 0], h2T_sb)

        # final: out[f] = LeakyReLU(bf[f] + max_n (hcat @ WfT)[n, f])
        for ft in range(FEAT // P):
            fsl = slice(ft * P, (ft + 1) * P)
            fm4 = work.tile([P, 4], F32, tag="fm")
            for nj in range(4):
                msl = slice(nj * 512, (nj + 1) * 512)
                pf = spsum.tile([P, 512], F32, tag="sp")
                for ct in range(4):
                    nc.tensor.matmul(
                        out=pf,
                        lhsT=wfT_sb[:, ct, fsl],
                        rhs=hcat[:, ct, msl],
                        start=(ct == 0),
                        stop=(ct == 3),
                    )
                nc.vector.tensor_reduce(
                    out=fm4[:, nj : nj + 1], in_=pf, axis=AX.X, op=ALU.max
                )
            m1 = work.tile([P, 1], F32, tag="m1")
            nc.vector.tensor_reduce(out=m1, in_=fm4, axis=AX.X, op=ALU.max)
            nc.vector.tensor_add(m1, m1, bf_sb[:, ft : ft + 1])
            t2 = work.tile([P, 1], F32, tag="t2")
            nc.scalar.activation(t2, m1, ACTF.Copy, scale=SLOPE)
            oc = work.tile([P, 1], F32, tag="oc")
            nc.vector.tensor_tensor(oc, m1, t2, op=ALU.max)
            nc.sync.dma_start(out[:, ft : ft + 1], oc)

    nc.compile()
    return nc


def make_in_maps(x, W1, b1, W2, b2, W3, b3, W4, b4, Wf, bf):
    """Host-side prep: shard x over batch, pre-transpose weights."""
    f32 = np.float32
    Ws = [W1, W2, W3, W4]
    bs = [b1, b2, b3, b4]
    shared = {}
    for li, (C, O) in enumerate(LAYERS):
        W = np.asarray(Ws[li], f32)
        Wd = W[:, :C]
        Wc = W[:, C:]
        shared[f"wd{li}"] = np.ascontiguousarray(Wd.T)
        shared[f"wv{li}"] = np.ascontiguousarray((Wc - Wd).T)
        shared[f"bias{li}"] = np.ascontiguousarray(np.asarray(bs[li], f32)[None, :])
    shared["wfT"] = np.ascontiguousarray(np.asarray(Wf, f32).T)
    shared["bf"] = np.ascontiguousarray(np.asarray(bf, f32).reshape(1, FEAT))
    in_maps = []
    for b in range(B):
        m = dict(shared)
        m["xT"] = np.ascontiguousarray(np.asarray(x[b], f32).T)
        in_maps.append(m)
    return in_maps


_NC_CACHE = {}


def kernel(x, W1, b1, W2, b2, W3, b3, W4, b4, Wf, bf):
    from concourse.bass_utils import run_bass_kernel_spmd

    if "nc" not in _NC_CACHE:
        _NC_CACHE["nc"] = build_kernel()
    nc = _NC_CACHE["nc"]
    in_maps = make_in_maps(x, W1, b1, W2, b2, W3, b3, W4, b4, Wf, bf)
    res = run_bass_kernel_spmd(nc, in_maps, core_ids=list(range(B)))
    out = np.stack([r["out"].T.ravel() for r in res.results])
    return out.astype(np.float32)


# revision 25
# speedup vs baseline: 415.1923x; 1.0197x over previous
"""DGCNN on Trainium2: data-parallel over batch B=8 across 8 NeuronCores.

Per core (one point cloud, N=2048):
  Each EdgeConv layer (C -> O) is decomposed as
    y[n,m]   = <h_n, h_m> - ||h_m||^2/2          (monotone in -dist^2, row-wise)
    topk_20  = 3 rounds of DVE max8/max_index/match_replace over y rows
    u        = h @ Wd^T              (Wd = W[:, :C], the (nbr-center) half)
    v        = h @ (Wc - Wd)^T + b   (Wc = W[:, C:])
    h'       = LeakyReLU(v_n + max_{m in knn(n)} u_m)
  using that max over neighbors commutes with the (monotone) LeakyReLU and
  with the per-point additive term v_n.  Neighbor u columns are fetched from
  SBUF-resident uT with gpsimd ap_gather (per-16-partition wrapped indices),
  then max-reduced on DVE — results land directly in transposed (O, N)
  layout for the next layer.
  Final layer: out = max_n LeakyReLU(hcat @ Wf^T + bf)  (max commutes again).
"""

import numpy as np

import concourse.bass as bass
import concourse.mybir as mybir
import concourse.tile as tile

F32 = mybir.dt.float32
U16 = mybir.dt.uint16
I16 = mybir.dt.int16
AX = mybir.AxisListType
ALU = mybir.AluOpType
ACTF = mybir.ActivationFunctionType

B, N, K = 8, 2048, 20
P = 128
NT = N // P  # 16 row tiles
LAYERS = [(3, 64), (64, 64), (64, 128), (128, 256)]
CSUM = 512
FEAT = 1024
SLOPE = 0.2
NEG_SENTINEL = -3.4028234663852886e38  # -FLT_MAX
NIDX = P * K  # 2560 gather indices per row tile (ap_gather only needs %4==0)


def build_kernel(gather_mode="ap"):
    from contextlib import ExitStack

    from concourse import bacc

    nc = bacc.Bacc("TRN2", target_bir_lowering=False, debug=False)

    xT = nc.dram_tensor("xT", (3, N), F32, kind="ExternalInput").ap()
    wd_d, wv_d, bias_d = [], [], []
    for li, (C, O) in enumerate(LAYERS):
        wd_d.append(nc.dram_tensor(f"wd{li}", (C, O), F32, kind="ExternalInput").ap())
        wv_d.append(nc.dram_tensor(f"wv{li}", (C, O), F32, kind="ExternalInput").ap())
        bias_d.append(
            nc.dram_tensor(f"bias{li}", (1, O), F32, kind="ExternalInput").ap()
        )
    wfT = nc.dram_tensor("wfT", (CSUM, FEAT), F32, kind="ExternalInput").ap()
    bf = nc.dram_tensor("bf", (1, FEAT), F32, kind="ExternalInput").ap()
    out = nc.dram_tensor("out", (P, FEAT // P), F32, kind="ExternalOutput").ap()

    with tile.TileContext(nc) as tc, ExitStack() as ctx:
        const = ctx.enter_context(tc.tile_pool(name="const", bufs=1))
        work = ctx.enter_context(tc.tile_pool(name="work", bufs=3))
        ypool = ctx.enter_context(tc.tile_pool(name="ypool", bufs=3))
        gpool = ctx.enter_context(tc.tile_pool(name="gpool", bufs=4))
        onepool = ctx.enter_context(tc.tile_pool(name="onepool", bufs=1))
        ypsum = ctx.enter_context(tc.tile_pool(name="ypsum", bufs=4, space="PSUM"))
        spsum = ctx.enter_context(tc.tile_pool(name="spsum", bufs=3, space="PSUM"))

        ones_col = const.tile([1, P], F32, tag="ones")
        nc.vector.memset(ones_col, 1.0)
        ones_row = const.tile([1, 512], F32, tag="onesr")
        nc.vector.memset(ones_row, 1.0)
        neghalf = const.tile([P, 1], F32, tag="neghalf")
        nc.vector.memset(neghalf, -0.5)

        # hcat layout: [128, 4, N]; channel c of hcat = hcat[c % 128, c // 128]
        hcat = const.tile([P, 4, N], F32, tag="hcat")
        xT_sb = const.tile([3, N], F32, tag="xT")
        nc.sync.dma_start(xT_sb, xT)
        wfT_sb = const.tile([P, 4, FEAT], F32, tag="wfT")
        nc.sync.dma_start(wfT_sb, wfT.rearrange("(t p) f -> p t f", p=P))
        bf_sb = const.tile([P, FEAT // P], F32, tag="bf")
        nc.sync.dma_start(bf_sb, bf.rearrange("o (t p) -> p (o t)", p=P))

        # h2 needs its own base-partition-0 tensor (PE matmul operands must
        # share base partition); it is DMA'd into hcat[64:128, 0] at the end.
        h2T_sb = const.tile([64, N], F32, tag="h2T")
        h_ins = [xT_sb, hcat[0:64, 0], h2T_sb, hcat[:, 1]]
        # per layer: list of (o0, o1, dest (O-chunk, N) AP) for the output
        h_outs = [
            [(0, 64, hcat[0:64, 0])],
            [(0, 64, h2T_sb)],
            [(0, 128, hcat[:, 1])],
            [(0, 128, hcat[:, 2]), (128, 256, hcat[:, 3])],
        ]

        for li, (C, O) in enumerate(LAYERS):
            hT = h_ins[li]  # (C, N)
            wd_sb = const.tile([C, O], F32, tag=f"wd{li}")
            nc.sync.dma_start(wd_sb, wd_d[li])
            wv_sb = const.tile([C, O], F32, tag=f"wv{li}")
            nc.sync.dma_start(wv_sb, wv_d[li])
            bias_sb = const.tile([1, O], F32, tag=f"bias{li}")
            nc.sync.dma_start(bias_sb, bias_d[li])

            # negxx[m] = -||h_m||^2 / 2, as a (1, N) row
            hsq = onepool.tile([C, N], F32, tag="hsq")
            nc.vector.tensor_tensor(hsq, hT, hT, op=ALU.mult)
            negxx = onepool.tile([1, N], F32, tag="negxx")
            for j in range(4):
                ps = spsum.tile([1, 512], F32, tag="sp")
                nc.tensor.matmul(
                    out=ps,
                    lhsT=neghalf[:C],
                    rhs=hsq[:, j * 512 : (j + 1) * 512],
                    start=True,
                    stop=True,
                )
                nc.scalar.copy(negxx[:, j * 512 : (j + 1) * 512], ps)

            # uT/vT: (O, N) SBUF, in <=128-row chunks; vT includes the bias.
            # Emission of m-chunk j is deferred into tile iteration j so the
            # PE/ACT work overlaps the first tiles' top-k instead of
            # serializing at the layer start (gather(0) still sees all of uT
            # complete well inside topk(0)'s window).
            chunks = h_outs[li]
            uT = []
            vT = []
            for ci, (o0, o1, _dest) in enumerate(chunks):
                W = o1 - o0
                uTc = onepool.tile([W, N], F32, tag=f"uT{ci}", name=f"uT{ci}")
                vTc = onepool.tile([W, N], F32, tag=f"vT{ci}", name=f"vT{ci}")
                uT.append(uTc)
                vT.append(vTc)

            def emit_uv_chunk(j):
                msl = slice(j * 512, (j + 1) * 512)
                for ci, (o0, o1, _dest) in enumerate(chunks):
                    W = o1 - o0
                    psu = spsum.tile([W, 512], F32, tag="sp")
                    nc.tensor.matmul(
                        out=psu, lhsT=wd_sb[:, o0:o1], rhs=hT[:, msl],
                        start=True, stop=True,
                    )
                    nc.scalar.copy(uT[ci][:, msl], psu)
                    psv = spsum.tile([W, 512], F32, tag="sp")
                    nc.tensor.matmul(
                        out=psv, lhsT=wv_sb[:, o0:o1], rhs=hT[:, msl],
                        start=True, stop=False,
                    )
                    nc.tensor.matmul(
                        out=psv, lhsT=bias_sb[:, o0:o1], rhs=ones_row,
                        start=False, stop=True,
                    )
                    nc.scalar.copy(vT[ci][:, msl], psv)

            def emit_reduce(t, gaths):
                """Neighbor-max + v + LeakyReLU for tile t (gaths: per chunk)."""
                nsl = slice(t * P, (t + 1) * P)
                for ci, (o0, o1, dest) in enumerate(chunks):
                    W = o1 - o0
                    gath = gaths[ci]
                    # gath[o, c*128 + n] -> reduce over c
                    red = work.tile([P, P], F32, tag="red")
                    nc.vector.tensor_reduce(
                        out=red[:W],
                        in_=gath[:W].rearrange("o (c n) -> o n c", n=P),
                        axis=AX.X,
                        op=ALU.max,
                    )
                    nc.vector.tensor_add(red[:W], red[:W], vT[ci][:, nsl])
                    tmp = work.tile([P, P], F32, tag="tmp")
                    nc.scalar.activation(tmp[:W], red[:W], ACTF.Copy, scale=SLOPE)
                    nc.vector.tensor_tensor(
                        dest[:, nsl], red[:W], tmp[:W], op=ALU.max
                    )

            pending = None  # deferred reduce stage: DVE consumes gath one tile late
            for t in range(NT):
                nsl = slice(t * P, (t + 1) * P)
                # y[n, m] = <h_n, h_m> - xx_m/2 for the 128 rows of this tile
                ybanks = []
                for j in range(4):
                    msl = slice(j * 512, (j + 1) * 512)
                    yb = ypsum.tile([P, 512], F32, tag="yb")
                    nc.tensor.matmul(
                        out=yb, lhsT=hT[:, nsl], rhs=hT[:, msl],
                        start=True, stop=False,
                    )
                    nc.tensor.matmul(
                        out=yb, lhsT=ones_col, rhs=negxx[:, msl],
                        start=False, stop=True,
                    )
                    ybanks.append(yb)
                y_sb = ypool.tile([P, N], F32, tag="y")
                for j in range(4):
                    nc.scalar.copy(y_sb[:, j * 512 : (j + 1) * 512], ybanks[j])
                if t == 0:
                    emit_uv_chunk(0)

                # top-20 neighbor indices by descending y (3 rounds of max8)
                idx24 = work.tile([P, 24], U16, tag="idx")
                for r in range(3):
                    vm = work.tile([P, 8], F32, tag="vmax")
                    nc.vector.max(out=vm, in_=y_sb)
                    nc.vector.max_index(
                        out=idx24[:, r * 8 : (r + 1) * 8], in_max=vm, in_values=y_sb
                    )
                    if r < 2:
                        nc.vector.match_replace(
                            out=y_sb, in_to_replace=vm, in_values=y_sb,
                            imm_value=NEG_SENTINEL,
                        )

                if t == 0:
                    # remaining uT/vT chunks must be emitted before the first
                    # gather so its read dependency covers all of uT
                    for j in range(1, 4):
                        emit_uv_chunk(j)

                # wrap first 20 indices for ap_gather: flat index i = c*128 + n,
                # stored at [partition i%16, slot i//16]; replicated to the Q7
                # core groups the gather uses.
                wrapped = work.tile([P, K * 8], I16, tag="wrap")
                if gather_mode == "zeroidx":
                    nc.vector.memset(wrapped, 0)
                else:
                    idx_i16 = idx24.bitcast(I16)
                    w3 = wrapped.rearrange("p (c r) -> p c r", r=8)
                    for r in range(8):
                        nc.sync.dma_start(
                            w3[0:16, :, r], idx_i16[16 * r : 16 * (r + 1), 0:K]
                        )
                    ngroups = max((o1 - o0) for (o0, o1, _d) in chunks) // 16
                    for g in range(1, ngroups):
                        nc.gpsimd.dma_start(
                            wrapped[16 * g : 16 * (g + 1), :], wrapped[0:16, :]
                        )

                # gather u columns of the 20 neighbors per chunk (Pool engine)
                gaths = []
                for ci, (o0, o1, _dest) in enumerate(chunks):
                    W = o1 - o0
                    gath = gpool.tile([P, NIDX], F32, tag="gath")
                    if gather_mode == "ap":
                        nc.gpsimd.ap_gather(
                            out_ap=gath[:W],
                            in_ap=uT[ci],
                            idxs_ap=wrapped[:W],
                            channels=W,
                            num_elems=N,
                            d=1,
                            num_idxs=NIDX,
                        )
                    else:  # bisect stub
                        nc.vector.memset(gath[:W], 0.0)
                    gaths.append(gath)

                # reduce stage runs one tile behind so the DVE never waits on
                # an in-flight gather at the head of its in-order queue
                if pending is not None:
                    emit_reduce(*pending)
                pending = (t, gaths)
            emit_reduce(*pending)

        # assemble h2 into hcat (partition-crossing move -> DMA)
        nc.sync.dma_start(hcat[64:128, 0], h2T_sb)

        # final: out[f] = LeakyReLU(bf[f] + max_n (hcat @ WfT)[n, f])
        for ft in range(FEAT // P):
            fsl = slice(ft * P, (ft + 1) * P)
            fm4 = work.tile([P, 4], F32, tag="fm")
            for nj in range(4):
                msl = slice(nj * 512, (nj + 1) * 512)
                pf = spsum.tile([P, 512], F32, tag="sp")
                for ct in range(4):
                    nc.tensor.matmul(
                        out=pf,
                        lhsT=wfT_sb[:, ct, fsl],
                        rhs=hcat[:, ct, msl],
                        start=(ct == 0),
                        stop=(ct == 3),
                    )
                nc.vector.tensor_reduce(
                    out=fm4[:, nj : nj + 1], in_=pf, axis=AX.X, op=ALU.max
                )
            m1 = work.tile([P, 1], F32, tag="m1")
            nc.vector.tensor_reduce(out=m1, in_=fm4, axis=AX.X, op=ALU.max)
            nc.vector.tensor_add(m1, m1, bf_sb[:, ft : ft + 1])
            t2 = work.tile([P, 1], F32, tag="t2")
            nc.scalar.activation(t2, m1, ACTF.Copy, scale=SLOPE)
            oc = work.tile([P, 1], F32, tag="oc")
            nc.vector.tensor_tensor(oc, m1, t2, op=ALU.max)
            nc.sync.dma_start(out[:, ft : ft + 1], oc)

    nc.compile()
    return nc


def make_in_maps(x, W1, b1, W2, b2, W3, b3, W4, b4, Wf, bf):
    """Host-side prep: shard x over batch, pre-transpose weights."""
    f32 = np.float32
    Ws = [W1, W2, W3, W4]
    bs = [b1, b2, b3, b4]
    shared = {}
    for li, (C, O) in enumerate(LAYERS):
        W = np.asarray(Ws[li], f32)
        Wd = W[:, :C]
        Wc = W[:, C:]
        shared[f"wd{li}"] = np.ascontiguousarray(Wd.T)
        shared[f"wv{li}"] = np.ascontiguousarray((Wc - Wd).T)
        shared[f"bias{li}"] = np.ascontiguousarray(np.asarray(bs[li], f32)[None, :])
    shared["wfT"] = np.ascontiguousarray(np.asarray(Wf, f32).T)
    shared["bf"] = np.ascontiguousarray(np.asarray(bf, f32).reshape(1, FEAT))
    in_maps = []
    for b in range(B):
        m = dict(shared)
        m["xT"] = np.ascontiguousarray(np.asarray(x[b], f32).T)
        in_maps.append(m)
    return in_maps


_NC_CACHE = {}


def kernel(x, W1, b1, W2, b2, W3, b3, W4, b4, Wf, bf):
    from concourse.bass_utils import run_bass_kernel_spmd

    if "nc" not in _NC_CACHE:
        _NC_CACHE["nc"] = build_kernel()
    nc = _NC_CACHE["nc"]
    in_maps = make_in_maps(x, W1, b1, W2, b2, W3, b3, W4, b4, Wf, bf)
    res = run_bass_kernel_spmd(nc, in_maps, core_ids=list(range(B)))
    out = np.stack([r["out"].T.ravel() for r in res.results])
    return out.astype(np.float32)
